# revision 11
# baseline (speedup 1.0000x reference)
"""Adaptive bilateral filter (nn_AdaptiveFilter) on 8 TRN2 NeuronCores.

Math: out_c(p) = sum_k x_c(p+d_k) * wt_k(p) / sum_k wt_k(p)
with wt_k = E[src(k)] * CF_k,  CF_k = exp(-50*(sum_c |g_c(p+d_k)-g_c(p)|)^2),
E = exp(w0) precomputed on HOST (slot-major bf16), src = reflect (7,7)->(4,4).

KEY STRUCTURE — tap symmetry: for the mirror tap k' = 48-k,
  CF_{k'}(p) = CF_k(p - v_k)   (v_k = tap offset vector)
so the color field (sub/abs/channel-sum/exp chain — the dominant cost) is
computed for only 24 pairs + center (25 of 49 taps).  Fields are computed
515 cols wide (one-sided extension) so column shifts are free AP offsets;
row shifts (impossible for lockstep engines) are realized by SBUF->SBUF
DMAs into a shifted-chunk tile CFS, with the 1-3 missing bottom rows per
pair ("slivers") batch-computed from DMA-gathered halo rows.

Sharding: 8 cores = 2 batches x 4 row-bands of 128 rows, halo included in
each core's DRAM band (134 x 3*518 bf16, channels interleaved per row).
No collectives.

Engines: DVE: subtract (4 tap-rows), wt = CF*E, prod = x*wt;  ACT: Abs,
Derivative_Erf(sqrt(50)*s) = (2/sqrt(pi))exp(-50 s^2) from PSUM (constant
cancels in num/den);  PE: channel-sum + den/num accumulation (identity
stationary);  DMA: sync queue.
"""
import sys
sys.path.insert(0, "/opt/trn_rl_repo")
import math
import numpy as np

import concourse.bacc as bacc
import concourse.mybir as mybir
import concourse.tile as tile
from concourse.ap import AP
from concourse.bass_utils import run_bass_kernel_spmd

F32 = mybir.dt.float32
BF16 = mybir.dt.bfloat16
AF = mybir.ActivationFunctionType
OP = mybir.AluOpType

KH = KW = 7
HB = 128           # band rows
W = 512
WP = 518           # padded width per channel
CWIDTH = 3 * WP    # 1554
WF = 515           # uniform field width (cc_rel in [0,515))
CS = 516           # CF chunk stride
CW = 7 * WF        # u tile per-channel stride (3605)
WJ = KW * W        # 3584
SCALE = math.sqrt(50.0)

_CACHE = {}
DBG_WT = None   # tap-row i: dump wt[:, DBG_J0*W:(DBG_J0+3)*W] to out
DBG_J0 = 0


def _fidx(i, j):
    return 7 * i + j if i < 3 else 21 + j     # (3,3) center = 24


# sliver slots: i-major, then field-jj, rr innermost (contiguous per field)
SLIV_BASE = {}
_s = 0
for _i in range(3):
    for _j in range(7):
        SLIV_BASE[(_i, _j)] = _s
        _s += 3 - _i
NSLIV = _s  # 42


def _v(t, dims, off=0):
    """AP keeping t's partition dim with custom free [stride, size] dims."""
    b = t[:] if not isinstance(t, AP) else t
    return AP(tensor=b.tensor, offset=b.offset + off,
              ap=[list(b.ap[0])] + [list(d) for d in dims])


def _emit(nc, tc, constp, gxp, workp, finp, psump, g_d, x_d, e_d, id_d,
          out_d):
    state = {"first": True}

    def mm(*args, **kwargs):
        inst = nc.tensor.matmul(*args, **kwargs)
        if state["first"]:
            state["first"] = False
        else:
            inst.ins.ldweights = False
        return inst

    ident = constp.tile([128, 128], BF16, tag="ident", name="ident")
    nc.sync.dma_start(ident[:], id_d.ap()[:, :])

    E = [constp.tile([HB, 4 * W], BF16, tag=f"E{t}", name=f"E{t}")
         for t in range(4)]
    for t in range(4):
        nc.sync.dma_start(E[t][:], e_d.ap()[:, t * 4 * W:(t + 1) * 4 * W])

    gt3 = constp.tile([HB, CWIDTH], BF16, tag="gt3", name="gt3")
    nc.sync.dma_start(gt3[:], g_d.ap()[3:3 + HB, :])

    CF = constp.tile([HB, 25 * CS], BF16, tag="CF", name="CF")
    CFS2 = constp.tile([HB, 3 * 7 * CS], BF16, tag="CFS2", name="CFS2")
    gA = constp.tile([NSLIV, 3 * W], BF16, tag="gA", name="gA")
    gB = constp.tile([NSLIV, 3 * W], BF16, tag="gB", name="gB")
    sCF = constp.tile([NSLIV, W], BF16, tag="sCF", name="sCF")
    uS = constp.tile([NSLIV, 3 * W], BF16, tag="uS", name="uS")

    # center tap: s == 0, and Derivative_Erf carries a 2/sqrt(pi) factor
    # that must be uniform across taps to cancel in num/den
    nc.gpsimd.memset(CF[:, 24 * CS:25 * CS], 2.0 / math.sqrt(math.pi))

    # sliver halo gathers: pair (i,jj) sliver row rr (rr < 3-i) needs
    #   gA: gp(q+v) = g_d row 128+i+rr, col offset 3   (jj-independent)
    #   gB: gp(q)   = g_d row 131+rr,   col offset 6-jj
    for i in range(3):
        nr = 3 - i
        base = SLIV_BASE[(i, 0)]
        bA = g_d.ap()[128 + i:128 + i + nr, :]
        nc.sync.dma_start(
            gA[base:base + 7 * nr, :],
            AP(tensor=bA.tensor, offset=bA.offset + 3,
               ap=[[0, 7], list(bA.ap[0]), [WP, 3], [1, W]]))
        for j in range(7):
            s0 = SLIV_BASE[(i, j)]
            bB = g_d.ap()[131:131 + nr, :]
            nc.sync.dma_start(
                gB[s0:s0 + nr, :],
                AP(tensor=bB.tensor, offset=bB.offset + 6 - j,
                   ap=[list(bB.ap[0]), [WP, 3], [1, W]]))

    den_ps = psump.tile([HB, W], F32, tag="dps", name="dps", bufs=1)
    num_wide = psump.tile([HB, 3 * W], F32, tag="npsw", name="npsw",
                          bufs=1)

    udict = {}
    wtdict = {}

    def emit_sub(i):
        if i < 3:
            gt = gxp.tile([HB, CWIDTH], BF16, tag="gt", name="gt", bufs=2)
            nc.sync.dma_start(gt[:], g_d.ap()[i:i + HB, :])
        else:
            gt = gt3
        u = workp.tile([HB, 3 * CW], BF16, tag="u", name="u", bufs=2)
        # fields j in [0,3): patch g_d col offset j (stride 1/j), center off 3
        nc.vector.tensor_tensor(
            _v(u, [[CW, 3], [WF, 3], [1, WF]]),
            _v(gt, [[WP, 3], [1, 3], [1, WF]]),
            _v(gt3, [[WP, 3], [0, 3], [1, WF]], 3),
            OP.subtract)
        if i < 3:
            # fields j in [3,7): patch offset 3 const, center offset 3..0
            nc.vector.tensor_tensor(
                _v(u, [[CW, 3], [WF, 4], [1, WF]], 3 * WF),
                _v(gt, [[WP, 3], [0, 4], [1, WF]], 3),
                _v(gt3, [[WP, 3], [-1, 4], [1, WF]], 3),
                OP.subtract)
        udict[i] = u

    def emit_abs(i):
        u = udict[i]
        a = _v(u, [[CW, 3], [1, 3 * WF]])
        nc.scalar.activation(a, a, AF.Abs)
        if i < 3:
            b = _v(u, [[CW, 3], [1, 4 * WF]], 3 * WF)
            nc.scalar.activation(b, b, AF.Abs)

    def emit_csum(i):
        u = udict.pop(i)
        for j in range(7 if i < 3 else 3):
            f = _fidx(i, j)
            sA = psump.tile([HB, W], F32, tag="sA", name="sA", bufs=2)
            for c in range(3):
                mm(sA[:], ident[:],
                   u[:, c * CW + j * WF:c * CW + j * WF + W],
                   start=(c == 0), stop=(c == 2))
            sB = psump.tile([HB, 8], F32, tag="sB", name="sB", bufs=2)
            for c in range(3):
                mm(sB[:, 0:3], ident[:],
                   u[:, c * CW + j * WF + W:c * CW + (j + 1) * WF],
                   start=(c == 0), stop=(c == 2))
            nc.scalar.activation(CF[:, f * CS:f * CS + W], sA[:],
                                 AF.Derivative_Erf, scale=SCALE)
            nc.scalar.activation(CF[:, f * CS + W:f * CS + WF], sB[:, 0:3],
                                 AF.Derivative_Erf, scale=SCALE)

    def emit_shift(ic):
        """Copy field row i = 6-ic, partition-shifted, into CFS2 block
        ic-4 (contiguous src/dst: fast DMA), stitch sliver rows below."""
        i = 6 - ic
        npart = 131 - ic
        nr = 3 - i
        B = (ic - 4) * 7 * CS
        nc.sync.dma_start(
            CFS2[0:npart, B:B + 7 * CS],
            CF[ic - 3:HB, (7 * i) * CS:(7 * i + 7) * CS])
        # sliver rows: field jj lands at chunk col jj*CS + max(0, 3-jj)
        for j in range(7):
            jj = 6 - j
            s0 = SLIV_BASE[(i, jj)]
            dc = B + jj * CS + max(0, 3 - jj)
            nc.sync.dma_start(CFS2[npart:HB, dc:dc + W],
                              sCF[s0:s0 + nr, :])

    def emit_wt(i):
        ri = min(i, 6 - i)
        eb = E[ri][:]
        wt = workp.tile([HB, WJ], BF16, tag="wt", name="wt", bufs=2)
        splits = [(0, 4), (4, 3)] if i < 3 else [(0, 3), (3, 4)]
        for (j0, nj) in splits:
            if i < 3:
                cfv = (_v(CF, [[CS, nj], [1, W]], (7 * i) * CS) if j0 == 0
                       else _v(CF, [[CS + 1, nj], [1, W]],
                               (7 * i + 4) * CS + 1))
            elif i == 3:
                cfv = (_v(CF, [[CS, nj], [1, W]], 21 * CS) if j0 == 0
                       else _v(CF, [[-(CS - 1), nj], [1, W]], 24 * CS))
            else:
                B = (i - 4) * 7 * CS
                cfv = (_v(CFS2, [[-CS, nj], [1, W]], B + 6 * CS) if j0 == 0
                       else _v(CFS2, [[-(CS - 1), nj], [1, W]], B + 3 * CS))
            if j0 == 0:
                ev = _v(eb, [[W, nj], [1, W]])
            else:
                # rj = min(j,6-j) descends from min(j0, 6-j0)
                ev = _v(eb, [[-W, nj], [1, W]], min(j0, 6 - j0) * W)
            nc.vector.tensor_tensor(
                _v(wt, [[W, nj], [1, W]], j0 * W), cfv, ev, OP.mult)
        wtdict[i] = wt
        if DBG_WT == i:
            nc.sync.dma_start(out_d.ap()[:, :],
                              wt[:, DBG_J0 * W:(DBG_J0 + 3) * W])

    def emit_cons(i):
        wt = wtdict.pop(i)
        xt = gxp.tile([HB, CWIDTH], BF16, tag="xt", name="xt", bufs=2)
        nc.sync.dma_start(xt[:], x_d.ap()[i:i + HB, :])
        first_i, last_i = (i == 0), (i == 6)
        for j in range(KW):
            mm(den_ps[:], ident[:], wt[:, j * W:(j + 1) * W],
               start=(first_i and j == 0), stop=(last_i and j == 6))
        prod = workp.tile([HB, 3 * WJ], BF16, tag="pr", name="pr", bufs=2)
        nc.vector.tensor_tensor(
            prod[:].rearrange("p (c n w) -> p c n w", c=3, n=KW),
            _v(xt, [[WP, 3], [1, KW], [1, W]]),
            _v(wt, [[0, 3], [W, KW], [1, W]]),
            OP.mult)
        for c in range(3):
            for j in range(KW):
                mm(num_wide[:, c * W:(c + 1) * W], ident[:],
                   prod[:, c * WJ + j * W:c * WJ + (j + 1) * W],
                   start=(first_i and j == 0), stop=(last_i and j == 6))

    # ---- schedule ----
    emit_sub(0)
    nc.vector.tensor_tensor(uS[:], gA[:], gB[:], OP.subtract)
    nc.scalar.activation(uS[:], uS[:], AF.Abs)
    emit_sub(1)
    emit_abs(0)
    psS = psump.tile([HB, W], F32, tag="sA", name="psS", bufs=2)
    for c in range(3):
        mm(psS[0:NSLIV, :], ident[0:NSLIV, 0:NSLIV],
           uS[:, c * W:(c + 1) * W], start=(c == 0), stop=(c == 2))
    nc.scalar.activation(sCF[:], psS[0:NSLIV, :], AF.Derivative_Erf,
                         scale=SCALE)
    emit_sub(2)
    emit_abs(1)
    emit_csum(0)
    emit_sub(3)
    emit_abs(2)
    emit_shift(6)
    emit_wt(0)
    emit_cons(0)
    emit_csum(1)
    emit_abs(3)
    emit_shift(5)
    emit_wt(1)
    emit_cons(1)
    emit_csum(2)
    emit_shift(4)
    emit_wt(2)
    emit_cons(2)
    emit_csum(3)
    for i in range(3, 7):
        emit_wt(i)
        emit_cons(i)

    rec = finp.tile([HB, W], F32, tag="rec", name="rec")
    # den in [~4e-3, ~60]: approx_fast's ~51 ULP is negligible vs bf16 noise
    nc.vector.reciprocal_approx_fast(rec[:], den_ps[:])
    o = finp.tile([HB, 3 * W], BF16, tag="o", name="o")
    nc.vector.tensor_tensor(
        o[:].rearrange("p (c w) -> p c w", c=3),
        num_wide[:].rearrange("p (c w) -> p c w", c=3),
        _v(rec, [[0, 3], [1, W]]), OP.mult)
    if DBG_WT is None:
        nc.sync.dma_start(out_d.ap()[:, :], o[:])


def _build():
    nc = bacc.Bacc("TRN2", target_bir_lowering=False, debug=False)
    g_d = nc.dram_tensor("g", [134, CWIDTH], BF16, kind="ExternalInput")
    x_d = nc.dram_tensor("x", [134, CWIDTH], BF16, kind="ExternalInput")
    e_d = nc.dram_tensor("e", [HB, 16 * W], BF16, kind="ExternalInput")
    id_d = nc.dram_tensor("ident", [128, 128], BF16, kind="ExternalInput")
    out_d = nc.dram_tensor("out", [HB, 3 * W], BF16,
                           kind="ExternalOutput")

    with tile.TileContext(nc) as tc:
        with (
            tc.tile_pool(name="const", bufs=1) as constp,
            tc.tile_pool(name="gx", bufs=2) as gxp,
            tc.tile_pool(name="work", bufs=2) as workp,
            tc.tile_pool(name="fin", bufs=1) as finp,
            tc.tile_pool(name="psum", bufs=1, space="PSUM") as psump,
        ):
            _emit(nc, tc, constp, gxp, workp, finp, psump,
                  g_d, x_d, e_d, id_d, out_d)

    nc.compile()
    return nc


def _shard_inputs(x, guidance, w0):
    import ml_dtypes
    BF = ml_dtypes.bfloat16
    pad = ((0, 0), (0, 0), (3, 3), (3, 3))
    # (B,3,518,518) -> per-core rows with channels interleaved per row
    xp = np.pad(x, pad, mode="reflect").astype(BF).transpose(0, 2, 1, 3)
    gp = np.pad(guidance, pad, mode="reflect").astype(BF).transpose(0, 2, 1, 3)
    ident = np.eye(128, dtype=BF)

    in_maps = []
    for c in range(8):
        b, band = divmod(c, 4)
        r0 = band * HB
        wslice = w0[b, r0 * W:(r0 + HB) * W]          # (65536, 4, 4)
        e = np.exp(wslice.reshape(HB, W, 4, 4).transpose(0, 2, 3, 1))
        in_maps.append({
            "g": np.ascontiguousarray(
                gp[b, r0:r0 + HB + 6].reshape(HB + 6, CWIDTH)),
            "x": np.ascontiguousarray(
                xp[b, r0:r0 + HB + 6].reshape(HB + 6, CWIDTH)),
            "e": np.ascontiguousarray(e.reshape(HB, 16 * W)).astype(BF),
            "ident": ident,
        })
    return in_maps


def kernel(x, guidance, w0):
    x = np.asarray(x, dtype=np.float32)
    guidance = np.asarray(guidance, dtype=np.float32)
    w0 = np.asarray(w0, dtype=np.float32)
    B, C, H, Wf = x.shape

    if "nc" not in _CACHE:
        _CACHE["nc"] = _build()
    nc = _CACHE["nc"]

    in_maps = _shard_inputs(x, guidance, w0)
    res = run_bass_kernel_spmd(nc, in_maps, core_ids=list(range(8)))

    out = np.empty((B, C, H, Wf), dtype=np.float32)
    for c in range(8):
        b, band = divmod(c, 4)
        r0 = band * HB
        blk = res.results[c]["out"].astype(np.float32).reshape(
            HB, 3, Wf).transpose(1, 0, 2)
        out[b, :, r0:r0 + HB, :] = blk
    return out


# revision 12
# speedup vs baseline: 1.5281x; 1.5281x over previous
"""Adaptive bilateral filter (nn_AdaptiveFilter) on 8 TRN2 NeuronCores.

Math: out_c(p) = sum_k x_c(p+d_k) * wt_k(p) / sum_k wt_k(p)
with wt_k = softmax_k(w)(p) * exp(-50 * (sum_c |g_c(p+d_k) - g_c(p)|)^2).
Softmax normalization cancels in num/den, so wt_k = E[src(k)] * exp(-50*s^2)
with E = exp(w0) precomputed on HOST (slot-major bf16) and src = reflect
map (7,7)->(4,4).

Sharding: 8 cores = 2 batches x 4 row-bands of 128 rows. Host reflect-pads
to (518,518), converts to bf16 and interleaves channels per row, shipping
each core a (134, 3*518) band (halo included) of g and x, E [128, 16*512]
bf16 slot-major, and the center tile gc [128, 3*512] bf16. No collectives.
Row-interleaved channels make every DMA descriptor a 3108-byte contiguous
run (vs 1036 for planar) and need no kernel-side AP changes.

Engine split per tap-row i (j-packed over 7 column taps, c-packed over 3
channels):
  DVE:    ONE bf16 subtract [128, 3*7*512] (sliding-window in0 vs broadcast
          center in1), wt = col*E (+-512-stride E views), ONE product
          x*wt [128, 3*7*512]
  ACT:    in-place Abs on the subtract output, per tap one
          Derivative_Erf(sqrt(50)*s) = 2/sqrt(pi)*exp(-50 s^2) straight
          from PSUM (the 2/sqrt(pi) cancels between num and den)
  PE:     channel-sum of |d| into PSUM (identity matmuls), den/num
          accumulation over the 49 taps
  DMA:    all on the sync queue (compute-engine queues stall on buffer
          semaphores; gpsimd DMA is slow-path); E chunk t is issued in
          slot t, and x_i one slot after g_i, so the critical g0 load
          is never starved.
Emission is software-pipelined: slot k runs sub_k + abs_k | DErf_{k-1} |
wt/prod/den/num_{k-2}; abs_k leads the ACT queue so the abs->s-mm chain
never waits on PE's backlog.  Iteration 0 is emitted in j-halves and its
B/C stages are pulled one slot early to prime the pipeline.  Output is
one packed bf16 [128, 3*512] DMA (host casts up and de-interleaves).
"""
import sys
sys.path.insert(0, "/opt/trn_rl_repo")
import math
import numpy as np

import concourse.bacc as bacc
import concourse.mybir as mybir
import concourse.tile as tile
from concourse.ap import AP
from concourse.bass_utils import run_bass_kernel_spmd

F32 = mybir.dt.float32
U16 = mybir.dt.uint16
BF16 = mybir.dt.bfloat16
AF = mybir.ActivationFunctionType
OP = mybir.AluOpType

KH = KW = 7
H_BAND = 128
W = 512
WP = 518
WJ = KW * W        # 3584
CJ = 3 * WJ        # 10752
SCALE = math.sqrt(50.0)  # Square(sqrt(50)*s) = 50*s^2
PAIRS = ((0, 2), (2, 2), (4, 2), (6, 1))

_CACHE = {}


def _view(ap_obj, dims):
    """AP with the tile's partition dim plus the given free [stride, size]."""
    base = ap_obj.ap
    return AP(tensor=ap_obj.tensor, offset=ap_obj.offset,
              ap=[list(base[0])] + [list(d) for d in dims])


def _emit(nc, tc, constp, gxp, workp, finp, psump, g_d, x_d, e_d, gc_d,
          id_d, out_d):
    ident = constp.tile([128, 128], BF16, tag="ident", name="ident")
    nc.sync.dma_start(ident[:], id_d.ap()[:, :])

    gc = constp.tile([H_BAND, 3 * W], BF16, tag="gc", name="gc")
    nc.sync.dma_start(gc[:], gc_d.ap()[:, :])

    E = [constp.tile([H_BAND, 4 * W], BF16, tag=f"E{t}", name=f"E{t}")
         for t in range(4)]

    den_ps = psump.tile([H_BAND, W], F32, tag="dps", name="dps", bufs=1)
    num_wide = psump.tile([H_BAND, 3 * W], F32, tag="npsw", name="npsw",
                          bufs=1)

    stageA = {}
    stageB = {}

    def emit_A1(i):
        gt = gxp.tile([H_BAND, 3 * WP], BF16, tag="gt", name="gt", bufs=2)
        nc.sync.dma_start(gt[:], g_d.ap()[i:i + H_BAND, :])
        if i < 4:
            nc.sync.dma_start(E[i][:],
                              e_d.ap()[:, i * 4 * W:(i + 1) * 4 * W])
        # u[p, c, j, w] = gt[p, c*518 + j + w] - gc[p, c*512 + w]
        u = workp.tile([H_BAND, CJ], BF16, tag="u", name="u", bufs=3)
        nc.vector.tensor_tensor(
            u[:].rearrange("p (c n w) -> p c n w", c=3, n=KW),
            _view(gt[:], [[WP, 3], [1, KW], [1, W]]),
            _view(gc[:], [[W, 3], [0, KW], [1, W]]),
            OP.subtract)
        stageA[i] = (u, None)

    def emit_A2(i):
        u, _ = stageA[i]
        nc.scalar.activation(u[:], u[:], AF.Abs)

    def emit_B(i):
        u, _ = stageA.pop(i)
        # x_i isn't read until stage C (next slot): issuing its DMA here
        # keeps the early sync queue clear for g/E
        xt = gxp.tile([H_BAND, 3 * WP], BF16, tag="xt", name="xt", bufs=3)
        nc.sync.dma_start(xt[:], x_d.ap()[i:i + H_BAND, :])
        col = workp.tile([H_BAND, WJ], BF16, tag="col", name="col", bufs=3)
        for j in range(KW):
            s_ps = psump.tile([H_BAND, W], F32, tag="sps", name="sps",
                              bufs=4)
            for c in range(3):
                nc.tensor.matmul(
                    s_ps[:], ident[:],
                    u[:, c * WJ + j * W:c * WJ + (j + 1) * W],
                    start=(c == 0), stop=(c == 2))
            nc.scalar.activation(col[:, j * W:(j + 1) * W], s_ps[:],
                                 AF.Derivative_Erf, scale=SCALE)
        stageB[i] = (col, xt)

    def emit_C(i):
        col, xt = stageB.pop(i)
        ri = min(i, 6 - i)
        first_i, last_i = (i == 0), (i == 6)
        eb = E[ri][:]
        # wt = col * E(src tap): j in 0..3 reads E[ri] slots 0..3 (+W step),
        # j in 4..6 reads slots 2..0 (-W step)
        wt = workp.tile([H_BAND, WJ], BF16, tag="wt", name="wt", bufs=3)
        nc.vector.tensor_tensor(
            wt[:, 0:4 * W].rearrange("p (n w) -> p n w", n=4),
            col[:, 0:4 * W].rearrange("p (n w) -> p n w", n=4),
            _view(eb, [[W, 4], [1, W]]), OP.mult)
        nc.vector.tensor_tensor(
            wt[:, 4 * W:].rearrange("p (n w) -> p n w", n=3),
            col[:, 4 * W:].rearrange("p (n w) -> p n w", n=3),
            AP(tensor=eb.tensor, offset=eb.offset + 2 * W,
               ap=[list(eb.ap[0]), [-W, 3], [1, W]]), OP.mult)
        for j in range(KW):
            nc.tensor.matmul(den_ps[:], ident[:], wt[:, j * W:(j + 1) * W],
                             start=(first_i and j == 0),
                             stop=(last_i and j == 6))
        # prod[p, c, j, w] = xt[p, c*518 + j + w] * wt[p, j*512 + w]
        prod = workp.tile([H_BAND, CJ], BF16, tag="pr", name="pr", bufs=2)
        nc.vector.tensor_tensor(
            prod[:].rearrange("p (c n w) -> p c n w", c=3, n=KW),
            _view(xt[:], [[WP, 3], [1, KW], [1, W]]),
            _view(wt[:], [[0, 3], [W, KW], [1, W]]),
            OP.mult)
        for c in range(3):
            for j in range(KW):
                nc.tensor.matmul(
                    num_wide[:, c * W:(c + 1) * W], ident[:],
                    prod[:, c * WJ + j * W:c * WJ + (j + 1) * W],
                    start=(first_i and j == 0),
                    stop=(last_i and j == 6))

    # Slots 0..6: sub_k + abs_k | DErf_{k-1} | stage-C_{k-2}.  abs_k
    # leads the ACT queue so the abs->s-mm chain never waits on PE's
    # backlog; DErf_{k-1} absorbs the PE wait afterwards.
    def emit_A1_split0():
        gt = gxp.tile([H_BAND, 3 * WP], BF16, tag="gt", name="gt", bufs=2)
        nc.sync.dma_start(gt[:], g_d.ap()[0:H_BAND, :])
        nc.sync.dma_start(E[0][:], e_d.ap()[:, 0:4 * W])
        u = workp.tile([H_BAND, CJ], BF16, tag="u", name="u", bufs=3)
        for j0, nj in ((0, 4), (4, 3)):
            nc.vector.tensor_tensor(
                AP(tensor=u[:].tensor, offset=u[:].offset + j0 * W,
                   ap=[list(u[:].ap[0]), [WJ, 3], [W, nj], [1, W]]),
                AP(tensor=gt[:].tensor, offset=gt[:].offset + j0,
                   ap=[list(gt[:].ap[0]), [WP, 3], [1, nj], [1, W]]),
                AP(tensor=gc[:].tensor, offset=gc[:].offset,
                   ap=[list(gc[:].ap[0]), [W, 3], [0, nj], [1, W]]),
                OP.subtract)
            v = AP(tensor=u[:].tensor, offset=u[:].offset + j0 * W,
                   ap=[list(u[:].ap[0]), [WJ, 3], [1, nj * W]])
            nc.scalar.activation(v, v, AF.Abs)
        stageA[0] = (u, None)

    # Prologue primes the pipeline one slot early: B_0 runs in slot 0
    # (PE/ACT would otherwise idle there), C_0 in slot 1.
    emit_A1_split0()
    emit_B(0)
    emit_A1(1)
    emit_A2(1)
    emit_C(0)
    for i in range(2, KH):
        emit_A1(i)
        emit_A2(i)
        emit_B(i - 1)
        if i >= 3:
            emit_C(i - 2)
    emit_B(6)
    emit_C(5)
    emit_C(6)

    rec = finp.tile([H_BAND, W], F32, tag="rec", name="rec")
    # den in [~4e-3, ~60]: approx_fast's ~51 ULP is negligible vs bf16 noise
    nc.vector.reciprocal_approx_fast(rec[:], den_ps[:])
    o = finp.tile([H_BAND, 3 * W], BF16, tag="o", name="o")
    nc.vector.tensor_tensor(
        o[:].rearrange("p (c w) -> p c w", c=3),
        num_wide[:].rearrange("p (c w) -> p c w", c=3),
        _view(rec[:], [[0, 3], [1, W]]), OP.mult)
    nc.sync.dma_start(out_d.ap()[:, :], o[:])


def _build():
    nc = bacc.Bacc("TRN2", target_bir_lowering=False, debug=False)
    g_d = nc.dram_tensor("g", [134, 3 * WP], BF16, kind="ExternalInput")
    x_d = nc.dram_tensor("x", [134, 3 * WP], BF16, kind="ExternalInput")
    e_d = nc.dram_tensor("e", [H_BAND, 16 * W], BF16, kind="ExternalInput")
    gc_d = nc.dram_tensor("gc", [H_BAND, 3 * W], BF16, kind="ExternalInput")
    id_d = nc.dram_tensor("ident", [128, 128], BF16, kind="ExternalInput")
    out_d = nc.dram_tensor("out", [H_BAND, 3 * W], BF16,
                           kind="ExternalOutput")

    with tile.TileContext(nc) as tc:
        with (
            tc.tile_pool(name="const", bufs=1) as constp,
            tc.tile_pool(name="gx", bufs=2) as gxp,
            tc.tile_pool(name="work", bufs=2) as workp,
            tc.tile_pool(name="fin", bufs=1) as finp,
            tc.tile_pool(name="psum", bufs=1, space="PSUM") as psump,
        ):
            _emit(nc, tc, constp, gxp, workp, finp, psump,
                  g_d, x_d, e_d, gc_d, id_d, out_d)

    nc.compile()
    return nc


def _shard_inputs(x, guidance, w0):
    import ml_dtypes
    BF = ml_dtypes.bfloat16
    pad = ((0, 0), (0, 0), (3, 3), (3, 3))
    # (B,3,518,518) -> per-core rows with channels interleaved per row:
    # band[r, c*518 + w]
    xp = np.pad(x, pad, mode="reflect").astype(BF).transpose(0, 2, 1, 3)
    gp = np.pad(guidance, pad, mode="reflect").astype(BF).transpose(0, 2, 1, 3)
    ident = np.eye(128, dtype=BF)

    in_maps = []
    for c in range(8):
        b, band = divmod(c, 4)
        r0 = band * H_BAND
        wslice = w0[b, r0 * W:(r0 + H_BAND) * W]          # (65536, 4, 4)
        e = np.exp(wslice.reshape(H_BAND, W, 4, 4).transpose(0, 2, 3, 1))
        gcore = gp[b, 3 + r0:3 + r0 + H_BAND, :, 3:3 + W]  # (128, 3, 512)
        in_maps.append({
            "g": np.ascontiguousarray(
                gp[b, r0:r0 + H_BAND + 6].reshape(H_BAND + 6, 3 * WP)),
            "x": np.ascontiguousarray(
                xp[b, r0:r0 + H_BAND + 6].reshape(H_BAND + 6, 3 * WP)),
            "e": np.ascontiguousarray(e.reshape(H_BAND, 16 * W)).astype(BF),
            "gc": np.ascontiguousarray(gcore.reshape(H_BAND, 3 * W)),
            "ident": ident,
        })
    return in_maps


def kernel(x, guidance, w0):
    x = np.asarray(x, dtype=np.float32)
    guidance = np.asarray(guidance, dtype=np.float32)
    w0 = np.asarray(w0, dtype=np.float32)
    B, C, H, Wf = x.shape

    if "nc" not in _CACHE:
        _CACHE["nc"] = _build()
    nc = _CACHE["nc"]

    in_maps = _shard_inputs(x, guidance, w0)
    res = run_bass_kernel_spmd(nc, in_maps, core_ids=list(range(8)))

    out = np.empty((B, C, H, Wf), dtype=np.float32)
    for c in range(8):
        b, band = divmod(c, 4)
        r0 = band * H_BAND
        # (128, 3*512) bf16 -> (3, 128, 512) f32
        blk = res.results[c]["out"].astype(np.float32).reshape(
            H_BAND, 3, Wf).transpose(1, 0, 2)
        out[b, :, r0:r0 + H_BAND, :] = blk
    return out
